# revision 26
# baseline (speedup 1.0000x reference)
"""Multi-head attention (16 heads, S=4096, D=1024) on 8 TRN2 NeuronCores.

Megatron-style tensor parallelism over heads: core i owns heads (2i, 2i+1).
Each core computes its head slice of the q/k/v projections, full attention
for its 2 heads (writing the softmax probabilities, which are part of the
module output), and a rank-128 partial of the output projection. The host
sums the 8 partials (the "all-reduce") and concatenates the attention
probability slices.

Device algorithm per core (matmuls bf16, softmax/normalization fp32):
  qhT[dh,s] = wq_slice @ q^T  (dh = 128 = 2 heads x 64); same khT, vhT.
  vh_aug[sk,65] = [vh | 1] per head via PE transpose.
  pass A (per sq-block j of 512):
    for sk-tile t: both heads' K^T-dot-Q as concurrent 64-row PE tiles into
    one [128,1024] PSUM pair -> one exp (ACT) -> per head
    ctx^T[65,512] += [vh|1]^T @ exp  (row 64 = softmax denominator)
    denominators -> 1/rowsum columns (DVE recip + PE transpose)
  pass B (per sq-tile jj of 128, lagged one j-block behind pass A):
    both heads' Q^T-dot-K as concurrent 64-row PE tiles -> one exp (ACT)
    into ab2[128, 2, S] -> attn = exp * (1/rowsum) (DVE) -> DMA
  partial[s,:] = sum_h (ctx_h^T @ woT_h) * (1/rowsum_h)[s], emitted per
  j-block so it overlaps the next block's attention work.
"""

import sys

sys.path.insert(0, "/opt/trn_rl_repo")

import ml_dtypes
import numpy as np

import concourse.mybir as mybir
from concourse import bacc
from concourse.masks import make_identity
from concourse.tile import TileContext

F32 = mybir.dt.float32
BF16 = mybir.dt.bfloat16
AF = mybir.ActivationFunctionType

D_MODEL = 1024
NUM_HEADS = 16
DEPTH = 64
TAU = 8.0
SEQ = 4096
N_CORES = 8
HPC = NUM_HEADS // N_CORES  # heads per core = 2
DH_SLICE = HPC * DEPTH  # 128 output dims per core


def build_mha_core(seq: int = SEQ, d_model: int = D_MODEL):
    """Build the per-core Bass module (same SPMD program on all 8 cores)."""
    S, D = seq, d_model
    KB = D // 128          # contraction blocks for projections
    NJ = S // 512          # sq blocks (pass A)
    NT = S // 128          # sk tiles (pass A) == sq tiles (pass B)
    NM = S // 512          # sk blocks (pass B)
    SCALE = 1.0 / TAU

    nc = bacc.Bacc("TRN2", target_bir_lowering=False)

    qT = nc.dram_tensor("qT", [D, S], BF16, kind="ExternalInput")
    kT = nc.dram_tensor("kT", [D, S], BF16, kind="ExternalInput")
    vT = nc.dram_tensor("vT", [D, S], BF16, kind="ExternalInput")
    wqT = nc.dram_tensor("wqT", [D, 128], BF16, kind="ExternalInput")
    wkT = nc.dram_tensor("wkT", [D, 128], BF16, kind="ExternalInput")
    wvT = nc.dram_tensor("wvT", [D, 128], BF16, kind="ExternalInput")
    bq = nc.dram_tensor("bq", [128, 1], F32, kind="ExternalInput")
    bk = nc.dram_tensor("bk", [128, 1], F32, kind="ExternalInput")
    bv = nc.dram_tensor("bv", [128, 1], F32, kind="ExternalInput")
    woT = nc.dram_tensor("woT", [128, D], BF16, kind="ExternalInput")
    attn_out = nc.dram_tensor("attn_out", [HPC, S, S], F32, kind="ExternalOutput")
    partial = nc.dram_tensor("partial", [S, D], F32, kind="ExternalOutput")

    with TileContext(nc) as tc:
        with (
            tc.tile_pool(name="consts", bufs=1) as consts,
            tc.tile_pool(name="persist", bufs=1) as persist,
            tc.tile_pool(name="xin", bufs=4) as xin,
            tc.tile_pool(name="expp", bufs=3) as expp,
            tc.tile_pool(name="attnb", bufs=2) as attnb,
            tc.tile_pool(name="outb", bufs=2) as outb,
            tc.tile_pool(name="rsp", bufs=2) as rsp,
            tc.tile_pool(name="ps_a", bufs=2, space="PSUM") as ps_a,
            tc.tile_pool(name="ps_b", bufs=1, space="PSUM") as ps_b,
            tc.tile_pool(name="ps_c", bufs=2, space="PSUM") as ps_c,
        ):
            # ---- constants -------------------------------------------------
            ident = consts.tile([128, 128], F32)
            make_identity(nc, ident)
            identb = consts.tile([128, 128], BF16)
            make_identity(nc, identb)

            w_sb = {}
            b_sb = {}
            for name, wdram, bdram in (
                ("q", wqT, bq),
                ("k", wkT, bk),
                ("v", wvT, bv),
            ):
                w = consts.tile([128, D], BF16, tag=f"w{name}")
                for kb in range(KB):
                    nc.sync.dma_start(
                        out=w[:, kb * 128 : (kb + 1) * 128],
                        in_=wdram[kb * 128 : (kb + 1) * 128, :],
                    )
                w_sb[name] = w
                b = consts.tile([128, 1], F32, tag=f"b{name}")
                nc.sync.dma_start(out=b, in_=bdram[:, :])
                b_sb[name] = b
            # woT rows for head h at partition base 0: [64, HPC, D]
            woT_sb = consts.tile([64, HPC, D], BF16)
            for h in range(HPC):
                nc.sync.dma_start(
                    out=woT_sb[:, h, :], in_=woT[h * DEPTH : (h + 1) * DEPTH, :]
                )

            # ---- persistent tensors ---------------------------------------
            qhT_sb = persist.tile([128, S], BF16)
            khT_sb = persist.tile([128, S], BF16)
            vhT_sb = persist.tile([128, S], BF16)
            vh_aug = persist.tile([128, HPC, NT, DEPTH + 1], BF16)
            ctxT_h = [
                persist.tile([64, S], BF16, tag=f"ctxT{h}", name=f"ctxT{h}")
                for h in range(HPC)
            ]
            # 1/rowsum per head as per-sq-partition columns, fp32
            recip_cols = persist.tile([128, HPC, NT], F32)

            # ---- phase 1: projections -> qhT/khT/vhT [128, S] -------------
            for name, xdram, dest in (
                ("k", kT, khT_sb),
                ("q", qT, qhT_sb),
                ("v", vT, vhT_sb),
            ):
                w = w_sb[name]
                for n in range(S // 512):
                    ps = ps_c.tile([128, 512], F32, tag="psc", name="ps")
                    for kb in range(KB):
                        xt = xin.tile([128, 512], BF16, tag="xin")
                        nc.sync.dma_start(
                            out=xt,
                            in_=xdram[
                                kb * 128 : (kb + 1) * 128, n * 512 : (n + 1) * 512
                            ],
                        )
                        nc.tensor.matmul(
                            ps,
                            lhsT=w[:, kb * 128 : (kb + 1) * 128],
                            rhs=xt,
                            start=(kb == 0),
                            stop=(kb == KB - 1),
                        )
                    nc.vector.tensor_scalar_add(
                        dest[:, n * 512 : (n + 1) * 512], ps, b_sb[name]
                    )

            # ---- phase 1.5: vh_aug = [vh | 1] per head --------------------
            for h in range(HPC):
                hs = slice(h * DEPTH, (h + 1) * DEPTH)
                for t in range(NT):
                    nc.gpsimd.memset(vh_aug[:, h, t, DEPTH : DEPTH + 1], 1.0)
                    pst = ps_c.tile([128, DEPTH], BF16, tag="psc", name="pst")
                    nc.tensor.transpose(
                        pst,
                        vhT_sb[hs, t * 128 : (t + 1) * 128],
                        identb[hs, hs],
                    )
                    nc.vector.tensor_copy(vh_aug[:, h, t, 0:DEPTH], pst)

            # ---- phase 2: attention ---------------------------------------
            ab2_tiles = {}

            def emit_passB_unit(jj, m):
                """QK^T + exp for both heads of sq-tile jj, sk-block m."""
                if m == 0:
                    ab2_tiles[jj] = attnb.tile(
                        [128, HPC, S], F32, tag="attn", name=f"ab{jj % 2}"
                    )
                ab2 = ab2_tiles[jj]
                pslg = ps_b.tile([128, 1024], F32, tag="lg", name="pslg")
                for h in range(HPC):
                    hs = slice(h * DEPTH, (h + 1) * DEPTH)
                    nc.tensor.matmul(
                        pslg[:, h * 512 : (h + 1) * 512],
                        lhsT=qhT_sb[hs, jj * 128 : (jj + 1) * 128],
                        rhs=khT_sb[hs, m * 512 : (m + 1) * 512],
                        start=True,
                        stop=True,
                    )
                nc.scalar.activation(
                    ab2[:, :, m * 512 : (m + 1) * 512],
                    pslg.rearrange("p (h n) -> p h n", h=HPC),
                    AF.Exp,
                    scale=SCALE,
                )
                if m == NM - 1:
                    ab2 = ab2_tiles.pop(jj)
                    for h in range(HPC):
                        nc.vector.tensor_scalar_mul(
                            ab2[:, h, :], ab2[:, h, :], recip_cols[:, h, jj : jj + 1]
                        )
                        nc.sync.dma_start(
                            out=attn_out[h, jj * 128 : (jj + 1) * 128, :],
                            in_=ab2[:, h, :],
                        )

            def passB_units(jprev):
                return [
                    (jj, m)
                    for jj in range(4 * jprev, 4 * jprev + 4)
                    for m in range(NM)
                ]

            for j in range(NJ):
                js = slice(j * 512, (j + 1) * 512)
                units = passB_units(j - 1) if j > 0 else []
                U = len(units)
                done = 0
                psc = [
                    ps_c.tile([DEPTH + 1, 512], F32, tag="psc", name=f"psc{h}")
                    for h in range(HPC)
                ]
                prev = None
                for t in range(NT):
                    # pass A QK^T: both heads as concurrent 64-row PE tiles
                    psl = ps_a.tile([128, 1024], F32, tag="lt", name="psl")
                    for h in range(HPC):
                        hs = slice(h * DEPTH, (h + 1) * DEPTH)
                        nc.tensor.matmul(
                            psl[:, h * 512 : (h + 1) * 512],
                            lhsT=khT_sb[hs, t * 128 : (t + 1) * 128],
                            rhs=qhT_sb[hs, js],
                            start=True,
                            stop=True,
                        )
                    if prev is not None:
                        pex, pt = prev
                        for h in range(HPC):
                            nc.tensor.matmul(
                                psc[h],
                                lhsT=vh_aug[:, h, pt, :],
                                rhs=pex[:, h * 512 : (h + 1) * 512],
                                start=(pt == 0),
                                stop=(pt == NT - 1),
                            )
                    ex = expp.tile([128, 1024], BF16, tag="exp")
                    nc.scalar.activation(ex, psl, AF.Exp, scale=SCALE)
                    prev = (ex, t)
                    # pass-B filler for the previous j-block
                    lim = (t + 1) * U // NT
                    while done < lim:
                        emit_passB_unit(*units[done])
                        done += 1
                pex, pt = prev
                for h in range(HPC):
                    nc.tensor.matmul(
                        psc[h],
                        lhsT=vh_aug[:, h, pt, :],
                        rhs=pex[:, h * 512 : (h + 1) * 512],
                        start=(pt == 0),
                        stop=(pt == NT - 1),
                    )
                # epilogue: ctx columns + 1/rowsum columns for this block
                for h in range(HPC):
                    nc.vector.tensor_copy(ctxT_h[h][:, js], psc[h][0:DEPTH, :])
                    rn = rsp.tile([65, 512], F32, tag="rn", name="rn")
                    nc.vector.reciprocal(
                        rn[64:65, :], psc[h][DEPTH : DEPTH + 1, :]
                    )
                    for c in range(4):
                        pst = ps_c.tile([128, 1], F32, tag="psc", name="pstr")
                        nc.tensor.transpose(
                            pst,
                            rn[64:65, c * 128 : (c + 1) * 128],
                            ident[64:65, 64:65],
                        )
                        nc.vector.tensor_copy(
                            recip_cols[:, h, j * 4 + c : j * 4 + c + 1], pst
                        )
                # output projection for this block's sq tiles
                for st in range(4 * j, 4 * j + 4):
                    ob = outb.tile([128, D], F32, tag="ob")
                    for dhf in range(D // 512):
                        dsl = slice(dhf * 512, (dhf + 1) * 512)
                        pso = []
                        for h in range(HPC):
                            p = ps_a.tile(
                                [128, 512], F32, tag="lt", name=f"pso{h}"
                            )
                            nc.tensor.matmul(
                                p,
                                lhsT=ctxT_h[h][:, st * 128 : (st + 1) * 128],
                                rhs=woT_sb[:, h, dsl],
                                start=True,
                                stop=True,
                            )
                            pso.append(p)
                        nc.vector.tensor_scalar_mul(
                            ob[:, dsl], pso[0], recip_cols[:, 0, st : st + 1]
                        )
                        nc.vector.scalar_tensor_tensor(
                            out=ob[:, dsl],
                            in0=pso[1],
                            scalar=recip_cols[:, 1, st : st + 1],
                            in1=ob[:, dsl],
                            op0=mybir.AluOpType.mult,
                            op1=mybir.AluOpType.add,
                        )
                    nc.sync.dma_start(
                        out=partial[st * 128 : (st + 1) * 128, :], in_=ob
                    )

            # tail: pass B for the last j-block
            for jj, m in passB_units(NJ - 1):
                emit_passB_unit(jj, m)

    nc.compile()
    return nc


def make_in_maps(q, k, v, wq_w, wq_b, wk_w, wk_b, wv_w, wv_b, wo_w):
    bf = ml_dtypes.bfloat16
    qT = np.ascontiguousarray(q.T).astype(bf)
    kT = np.ascontiguousarray(k.T).astype(bf)
    vT = np.ascontiguousarray(v.T).astype(bf)
    in_maps = []
    for i in range(N_CORES):
        sl = slice(i * DH_SLICE, (i + 1) * DH_SLICE)
        in_maps.append(
            {
                "qT": qT,
                "kT": kT,
                "vT": vT,
                "wqT": np.ascontiguousarray(wq_w[sl, :].T).astype(bf),
                "wkT": np.ascontiguousarray(wk_w[sl, :].T).astype(bf),
                "wvT": np.ascontiguousarray(wv_w[sl, :].T).astype(bf),
                "bq": np.ascontiguousarray(wq_b[sl].reshape(-1, 1), dtype=np.float32),
                "bk": np.ascontiguousarray(wk_b[sl].reshape(-1, 1), dtype=np.float32),
                "bv": np.ascontiguousarray(wv_b[sl].reshape(-1, 1), dtype=np.float32),
                "woT": np.ascontiguousarray(wo_w[:, sl].T).astype(bf),
            }
        )
    return in_maps


_NC_CACHE = {}


def _get_nc():
    if "nc" not in _NC_CACHE:
        _NC_CACHE["nc"] = build_mha_core()
    return _NC_CACHE["nc"]


def kernel(
    q,
    k,
    v,
    wq_w,
    wq_b,
    wk_w,
    wk_b,
    wv_w,
    wv_b,
    wo_w,
    wo_b,
    _trace: bool = False,
):
    from concourse.bass_utils import run_bass_kernel_spmd

    args = [np.asarray(x, dtype=np.float32) for x in (q, k, v)]
    wargs = [
        np.asarray(x, dtype=np.float32)
        for x in (wq_w, wq_b, wk_w, wk_b, wv_w, wv_b, wo_w)
    ]
    nc = _get_nc()
    in_maps = make_in_maps(*args, *wargs)
    res = run_bass_kernel_spmd(
        nc, in_maps, core_ids=list(range(N_CORES)), trace=_trace
    )
    out = np.zeros((SEQ, D_MODEL), np.float32)
    attn = np.empty((1, NUM_HEADS, SEQ, SEQ), np.float32)
    for i in range(N_CORES):
        out += res.results[i]["partial"]
        attn[0, i * HPC : (i + 1) * HPC] = res.results[i]["attn_out"]
    out += np.asarray(wo_b, np.float32)[None, :]
    out = out[None]  # [1, S, D]
    if _trace:
        kernel.last_results = res
    return out, attn


# revision 29
# speedup vs baseline: 1.1485x; 1.1485x over previous
"""Multi-head attention (16 heads, S=4096, D=1024) on 8 TRN2 NeuronCores.

Megatron-style tensor parallelism over heads: core i owns heads (2i, 2i+1).
Each core computes its head slice of the q/k/v projections, full attention
for its 2 heads (writing the softmax probabilities, which are part of the
module output), and a rank-128 partial of the output projection. The host
sums the 8 partials (the "all-reduce") and concatenates the attention
probability slices.

Device algorithm per core (matmuls bf16, softmax/normalization fp32):
  qhT[dh,s] = wq_slice @ q^T  (dh = 128 = 2 heads x 64); same khT, vhT.
  vh_aug[sk,65] = [vh | 1] per head via PE transpose.
  pass A (per sq-block j of 512):
    for sk-tile t: both heads' K^T-dot-Q as concurrent 64-row PE tiles into
    one [128,1024] PSUM pair -> one exp (ACT) -> per head
    ctx^T[65,512] += [vh|1]^T @ exp  (row 64 = softmax denominator)
    denominators -> 1/rowsum columns (DVE recip + PE transpose)
  pass B (per sq-tile jj of 128, lagged one j-block behind pass A):
    both heads' Q^T-dot-K as concurrent 64-row PE tiles -> one exp (ACT)
    into ab2[128, 2, S] -> attn = exp * (1/rowsum) (DVE) -> DMA
  partial[s,:] = sum_h (ctx_h^T @ woT_h) * (1/rowsum_h)[s], emitted per
  j-block so it overlaps the next block's attention work.
"""

import sys

sys.path.insert(0, "/opt/trn_rl_repo")

import ml_dtypes
import numpy as np

import concourse.mybir as mybir
from concourse import bacc
from concourse.masks import make_identity
from concourse.tile import TileContext

F32 = mybir.dt.float32
BF16 = mybir.dt.bfloat16
AF = mybir.ActivationFunctionType

D_MODEL = 1024
NUM_HEADS = 16
DEPTH = 64
TAU = 8.0
SEQ = 4096
N_CORES = 8
HPC = NUM_HEADS // N_CORES  # heads per core = 2
DH_SLICE = HPC * DEPTH  # 128 output dims per core


def build_mha_core(seq: int = SEQ, d_model: int = D_MODEL):
    """Build the per-core Bass module (same SPMD program on all 8 cores)."""
    S, D = seq, d_model
    KB = D // 128          # contraction blocks for projections
    NJ = S // 512          # sq blocks (pass A)
    NT = S // 128          # sk tiles (pass A) == sq tiles (pass B)
    NM = S // 512          # sk blocks (pass B)
    SCALE = 1.0 / TAU

    nc = bacc.Bacc("TRN2", target_bir_lowering=False)

    qT = nc.dram_tensor("qT", [D, S], BF16, kind="ExternalInput")
    kT = nc.dram_tensor("kT", [D, S], BF16, kind="ExternalInput")
    vT = nc.dram_tensor("vT", [D, S], BF16, kind="ExternalInput")
    wqT = nc.dram_tensor("wqT", [D, 128], BF16, kind="ExternalInput")
    wkT = nc.dram_tensor("wkT", [D, 128], BF16, kind="ExternalInput")
    wvT = nc.dram_tensor("wvT", [D, 128], BF16, kind="ExternalInput")
    bq = nc.dram_tensor("bq", [128, 1], F32, kind="ExternalInput")
    bk = nc.dram_tensor("bk", [128, 1], F32, kind="ExternalInput")
    bv = nc.dram_tensor("bv", [128, 1], F32, kind="ExternalInput")
    woT = nc.dram_tensor("woT", [128, D], BF16, kind="ExternalInput")
    attn_out = nc.dram_tensor("attn_out", [HPC, S, S], F32, kind="ExternalOutput")
    partial = nc.dram_tensor("partial", [S, D], F32, kind="ExternalOutput")

    with TileContext(nc) as tc:
        with (
            tc.tile_pool(name="consts", bufs=1) as consts,
            tc.tile_pool(name="persist", bufs=1) as persist,
            tc.tile_pool(name="xin", bufs=4) as xin,
            tc.tile_pool(name="expp", bufs=3) as expp,
            tc.tile_pool(name="attnb", bufs=4) as attnb,
            tc.tile_pool(name="outb", bufs=2) as outb,
            tc.tile_pool(name="rsp", bufs=2) as rsp,
            tc.tile_pool(name="ps_a", bufs=2, space="PSUM") as ps_a,
            tc.tile_pool(name="ps_b", bufs=2, space="PSUM") as ps_b,
            tc.tile_pool(name="ps_c", bufs=2, space="PSUM") as ps_c,
        ):
            # ---- constants -------------------------------------------------
            ident = consts.tile([128, 128], F32)
            make_identity(nc, ident)
            ones_row = consts.tile([65, DEPTH], F32)
            nc.vector.memset(ones_row[64:65, :], 1.0)
            identb = consts.tile([128, 128], BF16)
            make_identity(nc, identb)

            w_sb = {}
            b_sb = {}
            for name, wdram, bdram in (
                ("q", wqT, bq),
                ("k", wkT, bk),
                ("v", wvT, bv),
            ):
                w = consts.tile([128, D], BF16, tag=f"w{name}")
                for kb in range(KB):
                    nc.sync.dma_start(
                        out=w[:, kb * 128 : (kb + 1) * 128],
                        in_=wdram[kb * 128 : (kb + 1) * 128, :],
                    )
                w_sb[name] = w
                b = consts.tile([128, 1], F32, tag=f"b{name}")
                nc.sync.dma_start(out=b, in_=bdram[:, :])
                b_sb[name] = b
            # woT rows for head h at partition base 0: [64, HPC, D]
            woT_sb = consts.tile([64, HPC, D], BF16)
            for h in range(HPC):
                nc.sync.dma_start(
                    out=woT_sb[:, h, :], in_=woT[h * DEPTH : (h + 1) * DEPTH, :]
                )

            # ---- persistent tensors ---------------------------------------
            qhT_sb = persist.tile([128, S], BF16)
            khT_sb = persist.tile([128, S], BF16)
            vhT_sb = persist.tile([128, S], BF16)
            vh_aug = persist.tile([128, HPC, NT, DEPTH + 1], BF16)
            ctxT_h = [
                persist.tile([64, S], BF16, tag=f"ctxT{h}", name=f"ctxT{h}")
                for h in range(HPC)
            ]
            # 1/rowsum per head as per-sq-partition columns, fp32
            recip_cols = persist.tile([128, HPC, NT], F32)

            # ---- phase 1: projections -> qhT/khT/vhT [128, S] -------------
            for name, xdram, dest in (
                ("k", kT, khT_sb),
                ("q", qT, qhT_sb),
                ("v", vT, vhT_sb),
            ):
                w = w_sb[name]
                for n in range(S // 512):
                    ps = ps_c.tile([128, 512], F32, tag="psc", name="ps")
                    for kb in range(KB):
                        xt = xin.tile([128, 512], BF16, tag="xin")
                        nc.sync.dma_start(
                            out=xt,
                            in_=xdram[
                                kb * 128 : (kb + 1) * 128, n * 512 : (n + 1) * 512
                            ],
                        )
                        nc.tensor.matmul(
                            ps,
                            lhsT=w[:, kb * 128 : (kb + 1) * 128],
                            rhs=xt,
                            start=(kb == 0),
                            stop=(kb == KB - 1),
                        )
                    nc.vector.tensor_scalar_add(
                        dest[:, n * 512 : (n + 1) * 512], ps, b_sb[name]
                    )

            # ---- phase 1.5: vh_aug = [vh | 1] per head --------------------
            for h in range(HPC):
                hs = slice(h * DEPTH, (h + 1) * DEPTH)
                for t in range(NT):
                    nc.gpsimd.memset(vh_aug[:, h, t, DEPTH : DEPTH + 1], 1.0)
                    pst = ps_c.tile([128, DEPTH], BF16, tag="psc", name="pst")
                    nc.tensor.transpose(
                        pst,
                        vhT_sb[hs, t * 128 : (t + 1) * 128],
                        identb[hs, hs],
                    )
                    nc.vector.tensor_copy(vh_aug[:, h, t, 0:DEPTH], pst)

            # ---- phase 2: attention ---------------------------------------
            ab_tiles = {}
            ob_tiles = {}

            def emit_passB_unit(h, jj, m):
                """QK^T + exp for head h, sq-tile jj, sk-block m."""
                hs = slice(h * DEPTH, (h + 1) * DEPTH)
                if m == 0:
                    ab_tiles[(h, jj)] = attnb.tile(
                        [128, S], F32, tag="attn", name=f"ab{h}"
                    )
                ab = ab_tiles[(h, jj)]
                pslg = ps_b.tile([128, 512], F32, tag="lg", name="pslg")
                nc.tensor.matmul(
                    pslg,
                    lhsT=qhT_sb[hs, jj * 128 : (jj + 1) * 128],
                    rhs=khT_sb[hs, m * 512 : (m + 1) * 512],
                    start=True,
                    stop=True,
                )
                nc.scalar.activation(
                    ab[:, m * 512 : (m + 1) * 512],
                    pslg,
                    AF.Exp,
                    scale=SCALE,
                )
                if m == NM - 1:
                    ab = ab_tiles.pop((h, jj))
                    nc.vector.tensor_scalar_mul(
                        ab, ab, recip_cols[:, h, jj : jj + 1]
                    )
                    nc.sync.dma_start(
                        out=attn_out[h, jj * 128 : (jj + 1) * 128, :], in_=ab
                    )

            def emit_outproj_unit(st, dhf):
                """Output projection for sq-tile st, D-half dhf."""
                if dhf == 0:
                    ob_tiles[st] = outb.tile([128, D], F32, tag="ob", name="ob")
                ob = ob_tiles[st]
                dsl = slice(dhf * 512, (dhf + 1) * 512)
                pso = ps_b.tile([128, 512], F32, tag="lg", name="pso")
                for h in range(HPC):
                    nc.tensor.matmul(
                        pso,
                        lhsT=ctxT_h[h][:, st * 128 : (st + 1) * 128],
                        rhs=woT_sb[:, h, dsl],
                        start=(h == 0),
                        stop=(h == HPC - 1),
                    )
                nc.vector.tensor_copy(ob[:, dsl], pso)
                if dhf == D // 512 - 1:
                    ob = ob_tiles.pop(st)
                    nc.sync.dma_start(
                        out=partial[st * 128 : (st + 1) * 128, :], in_=ob
                    )

            def make_fillers(jprev):
                """Pass-B + out-proj work for block jprev, interleaved."""
                fill = []
                bunits = [
                    (h, jj, m)
                    for jj in range(4 * jprev, 4 * jprev + 4)
                    for m in range(NM)
                    for h in range(HPC)
                ]
                ounits = [
                    (st, dhf)
                    for st in range(4 * jprev, 4 * jprev + 4)
                    for dhf in range(D // 512)
                ]
                ob_every = max(1, len(bunits) // max(1, len(ounits)))
                oi = 0
                for i, u in enumerate(bunits):
                    fill.append(("B", u))
                    if (i + 1) % ob_every == 0 and oi < len(ounits):
                        fill.append(("O", ounits[oi]))
                        oi += 1
                while oi < len(ounits):
                    fill.append(("O", ounits[oi]))
                    oi += 1
                return fill

            def emit_filler(f):
                kind, u = f
                if kind == "B":
                    emit_passB_unit(*u)
                else:
                    emit_outproj_unit(*u)

            for j in range(NJ):
                js = slice(j * 512, (j + 1) * 512)
                fillers = make_fillers(j - 1) if j > 0 else []
                U = len(fillers)
                done = 0
                psc = [
                    ps_c.tile([DEPTH + 1, 512], F32, tag="psc", name=f"psc{h}")
                    for h in range(HPC)
                ]
                prev = None
                for t in range(NT):
                    # pass A QK^T: both heads as concurrent 64-row PE tiles
                    psl = ps_a.tile([128, 1024], F32, tag="lt", name="psl")
                    for h in range(HPC):
                        hs = slice(h * DEPTH, (h + 1) * DEPTH)
                        nc.tensor.matmul(
                            psl[:, h * 512 : (h + 1) * 512],
                            lhsT=khT_sb[hs, t * 128 : (t + 1) * 128],
                            rhs=qhT_sb[hs, js],
                            start=True,
                            stop=True,
                        )
                    if prev is not None:
                        pex, pt = prev
                        for h in range(HPC):
                            nc.tensor.matmul(
                                psc[h],
                                lhsT=vh_aug[:, h, pt, :],
                                rhs=pex[:, h * 512 : (h + 1) * 512],
                                start=(pt == 0),
                                stop=(pt == NT - 1),
                            )
                    ex = expp.tile([128, 1024], BF16, tag="exp")
                    nc.scalar.activation(ex, psl, AF.Exp, scale=SCALE)
                    prev = (ex, t)
                    # woven work for the previous j-block
                    lim = (t + 1) * U // NT
                    while done < lim:
                        emit_filler(fillers[done])
                        done += 1
                pex, pt = prev
                for h in range(HPC):
                    nc.tensor.matmul(
                        psc[h],
                        lhsT=vh_aug[:, h, pt, :],
                        rhs=pex[:, h * 512 : (h + 1) * 512],
                        start=(pt == 0),
                        stop=(pt == NT - 1),
                    )
                # epilogue: normalized ctx columns + 1/rowsum columns
                for h in range(HPC):
                    rn = rsp.tile([65, 512], F32, tag="rn", name="rn")
                    nc.vector.reciprocal(
                        rn[64:65, :], psc[h][DEPTH : DEPTH + 1, :]
                    )
                    psb = ps_a.tile([DEPTH, 512], F32, tag="lt", name="psb")
                    nc.tensor.matmul(
                        psb,
                        lhsT=ones_row[64:65, :],
                        rhs=rn[64:65, :],
                        start=True,
                        stop=True,
                    )
                    scl = rsp.tile([64, 512], F32, tag="scl", name="scl")
                    nc.vector.tensor_copy(scl, psb)
                    nc.vector.tensor_mul(
                        ctxT_h[h][:, js], psc[h][0:DEPTH, :], scl
                    )
                    for c in range(4):
                        pst = ps_c.tile([128, 1], F32, tag="psc", name="pstr")
                        nc.tensor.transpose(
                            pst,
                            rn[64:65, c * 128 : (c + 1) * 128],
                            ident[64:65, 64:65],
                        )
                        nc.vector.tensor_copy(
                            recip_cols[:, h, j * 4 + c : j * 4 + c + 1], pst
                        )

            # tail: woven work for the last j-block
            for f in make_fillers(NJ - 1):
                emit_filler(f)

    nc.compile()
    return nc


def make_in_maps(q, k, v, wq_w, wq_b, wk_w, wk_b, wv_w, wv_b, wo_w):
    bf = ml_dtypes.bfloat16
    qT = np.ascontiguousarray(q.T).astype(bf)
    kT = np.ascontiguousarray(k.T).astype(bf)
    vT = np.ascontiguousarray(v.T).astype(bf)
    in_maps = []
    for i in range(N_CORES):
        sl = slice(i * DH_SLICE, (i + 1) * DH_SLICE)
        in_maps.append(
            {
                "qT": qT,
                "kT": kT,
                "vT": vT,
                "wqT": np.ascontiguousarray(wq_w[sl, :].T).astype(bf),
                "wkT": np.ascontiguousarray(wk_w[sl, :].T).astype(bf),
                "wvT": np.ascontiguousarray(wv_w[sl, :].T).astype(bf),
                "bq": np.ascontiguousarray(wq_b[sl].reshape(-1, 1), dtype=np.float32),
                "bk": np.ascontiguousarray(wk_b[sl].reshape(-1, 1), dtype=np.float32),
                "bv": np.ascontiguousarray(wv_b[sl].reshape(-1, 1), dtype=np.float32),
                "woT": np.ascontiguousarray(wo_w[:, sl].T).astype(bf),
            }
        )
    return in_maps


_NC_CACHE = {}


def _get_nc():
    if "nc" not in _NC_CACHE:
        _NC_CACHE["nc"] = build_mha_core()
    return _NC_CACHE["nc"]


def kernel(
    q,
    k,
    v,
    wq_w,
    wq_b,
    wk_w,
    wk_b,
    wv_w,
    wv_b,
    wo_w,
    wo_b,
    _trace: bool = False,
):
    from concourse.bass_utils import run_bass_kernel_spmd

    args = [np.asarray(x, dtype=np.float32) for x in (q, k, v)]
    wargs = [
        np.asarray(x, dtype=np.float32)
        for x in (wq_w, wq_b, wk_w, wk_b, wv_w, wv_b, wo_w)
    ]
    nc = _get_nc()
    in_maps = make_in_maps(*args, *wargs)
    res = run_bass_kernel_spmd(
        nc, in_maps, core_ids=list(range(N_CORES)), trace=_trace
    )
    out = np.zeros((SEQ, D_MODEL), np.float32)
    attn = np.empty((1, NUM_HEADS, SEQ, SEQ), np.float32)
    for i in range(N_CORES):
        out += res.results[i]["partial"]
        attn[0, i * HPC : (i + 1) * HPC] = res.results[i]["attn_out"]
    out += np.asarray(wo_b, np.float32)[None, :]
    out = out[None]  # [1, S, D]
    if _trace:
        kernel.last_results = res
    return out, attn
